# revision 5
# baseline (speedup 1.0000x reference)
"""Conv2d 3x3 VALID stride-1 for Trainium2 (Bass/Tile), 8-core SPMD.
1-D Winograd F(2,3) along H: 12 matmuls per 8 output rows instead of 18.

x: [32, 128, 112, 112] f32, weight: [256, 128, 3, 3] f32
out: [32, 256, 110, 110] f32

Per output-row-pair t (55 pairs/image), with d = x rows [2t..2t+3]:
  t0 = d0-d2, t1 = d1+d2, t2 = d2-d1, t3 = d1-d3          (DVE, fp16, 2x)
  m_i = sum_kw Gw[:,kw,i,:] @ t_i[:, t, kw:kw+110]         (PE, 3 taps into PSUM bank i)
  o[2t]   = m0+m1+m2       o[2t+1] = m1-m2-m3              (DVE, all fp16)
m0..m3 live in two 2-bank PSUM tiles; ScalarE evacuates each pair with
one strided copy to fp16 SBUF as soon as its second bank stops, so the
PE never waits on PSUM recycling and the DVE never reads PSUM. Weight
transform Gw = G @ w over kh is precomputed on host; the fp16 output is
upcast to f32 on host. Data-parallel over batch: 4 img/core. H rows are
viewed as [56, 2] (pair, parity) so the stride-2 row reads of the
transform are plain slices.

Measured on 8xNC-v3 (axon): ~266 us NEFF exec on a cool chip (~320 us
when the chip is P0 power-throttled to 5/6 clock); matmul stream runs at
~185.5 ns per 440-free-dim fp16 matmul (~99% of the 183.3 ns roofline).
rel err (Frobenius) 6.1e-4 vs the fp32 jax reference.
"""

from collections import deque

import numpy as np

import concourse.mybir as mybir
import concourse.tile as tile
from concourse import bacc
from concourse.bass_utils import run_bass_kernel_spmd

B, CIN, H, W = 32, 128, 112, 112
COUT, KH, KW = 256, 3, 3
OH, OW = H - KH + 1, W - KW + 1  # 110, 110
NCORES = 8
BPC = B // NCORES  # images per core
HP = H // 2  # 56 row-pairs of input

NT = OH // 2            # 55 output row-pairs per image
TB = 4                  # row-pairs per block (PSUM bank: 4*110=440 <= 512)
BLOCKS = [(i * TB, TB) for i in range(NT // TB)] + [(NT - NT % TB, NT % TB)]
# -> 13 blocks of 4 + 1 block of 3
N_BCT = len(BLOCKS) * 2  # block-cts per image

# input-transform chunking: 8 chunks; a small first chunk (exactly the
# pairs block 0 needs) shortens the startup critical path. Chunk c covers
# output pairs [TCH[c]) and needs input row-pairs up to TCH[c][1]+1.
TCH = [(0, 4)] + [(8 * c - 4, min(8 * c + 4, NT)) for c in range(1, 8)]
# disjoint x DMA chunks (input row-pairs); chunk c covers what transform
# chunk c needs beyond chunk c-1
XCH = [(0, 5)] + [(8 * c - 3, min(8 * c + 5, HP)) for c in range(1, 8)]

# transform op i -> (pair_shift_a, parity_a, pair_shift_b, parity_b, op):
#   t_i[pair t] = x[2t + a] op x[2t + b],  row 2t+d -> (pair t + d//2, d%2)
_TOPS = [
    (0, 0, 1, 0, "subtract"),  # t0 = d0 - d2
    (0, 1, 1, 0, "add"),       # t1 = d1 + d2
    (1, 0, 0, 1, "subtract"),  # t2 = d2 - d1
    (0, 1, 1, 1, "subtract"),  # t3 = d1 - d3
]

F32 = mybir.dt.float32
FP16 = mybir.dt.float16

_CACHE = {}


def _build_nc():
    nc = bacc.Bacc("TRN2", target_bir_lowering=False, debug=False)
    OP = mybir.AluOpType

    x_d = nc.dram_tensor("x", [BPC, CIN, HP, 2, W], FP16, kind="ExternalInput")
    w_d = nc.dram_tensor("w", [CIN, KW, 4, COUT], FP16, kind="ExternalInput")
    # [.., 55, 2, 110]: output row 2t+p lives at [t, p] -> even/odd stores
    # are plain slices
    o_d = nc.dram_tensor("o", [BPC, COUT, NT, 2, OW], FP16, kind="ExternalOutput")

    from concourse.bass import _add_dep_helper

    with tile.TileContext(nc) as tc:
        with (
            tc.tile_pool(name="wpool", bufs=1) as wpool,
            tc.tile_pool(name="xpool", bufs=2) as xpool,
            tc.tile_pool(name="tpool", bufs=2) as tpool,
            tc.tile_pool(name="cpool", bufs=2) as cpool,
            tc.tile_pool(name="opool", bufs=8) as opool,
            tc.tile_pool(name="psum", bufs=8, space="PSUM") as psum,
        ):
            # PE pre-warm on garbage so the HAM clock gate is at 2.4 GHz
            # by the time real matmuls start.
            wscr = wpool.tile([128, 128], FP16, name="warm_w")
            xscr = wpool.tile([128, 4, 110], FP16, name="warm_x")
            nc.vector.memset(wscr[:], 0)
            nc.vector.memset(xscr[:], 0)
            ps_warm = psum.tile([128, 2, 4, 128], F32, name="warm_psum", tag="psA", bufs=2)
            for _ in range(16):
                nc.tensor.matmul(
                    ps_warm[:, 0, 0:4, 0:110], wscr[:], xscr[:],
                    start=True, stop=True, skip_group_check=True,
                )

            wr = wpool.tile([CIN, KW, 4, COUT], FP16)
            xts = [xpool.tile([CIN, HP, 2, W], FP16, tag="x", name="x0")]
            tts = [tpool.tile([CIN, 4, NT, W], FP16, tag="t", name="t0")]
            # x chunk 0 first: it gates the first transform ops + matmuls
            p0, p1 = XCH[0]
            nc.gpsimd.dma_start(xts[0][:, p0:p1, :, :], x_d[0, :, p0:p1, :, :])
            nc.gpsimd.dma_start(wr[:, :, :, 0:128], w_d[:, :, :, 0:128])
            nc.gpsimd.dma_start(wr[:, :, :, 128:256], w_d[:, :, :, 128:256])
            for p0, p1 in XCH[1:]:
                nc.gpsimd.dma_start(
                    xts[0][:, p0:p1, :, :], x_d[0, :, p0:p1, :, :]
                )

            def temit(b, j):
                """Input-transform op j (chunk j//4, i=j%4) for image b."""
                c, i = divmod(j, 4)
                t0, t1 = TCH[c]
                xr, tr = xts[b], tts[b]
                sa, pa, sb, pb, opname = _TOPS[i]
                nc.vector.tensor_tensor(
                    tr[:, i, t0:t1, :],
                    xr[:, t0 + sa : t1 + sa, pa, :],
                    xr[:, t0 + sb : t1 + sb, pb, :],
                    getattr(OP, opname),
                )

            # pending transform ops: (image, op j, min global bct to issue)
            NOPS = 4 * len(TCH)
            pending = deque()
            for j in range(12):
                temit(0, j)  # front-loaded; blocks 0..4 covered
            for j in range(12, NOPS):
                pending.append((0, j, j - 12))
            for b in range(1, BPC):
                for j in range(NOPS):
                    # x chunk j//4 of image b released at global bct
                    # (b-1)*N_BCT + 3*(j//4); +2 bcts for the DMA to land.
                    # Spread ops evenly (~1.15/bct): two transform ops in
                    # one bct puts the DVE over the PE period and the lag
                    # chains into PE stalls via the c-tile WAR.
                    el = max(3 * (j // 4) + 2, 2 + (25 * j) // 31)
                    pending.append((b, j, (b - 1) * N_BCT + el))

            gbct = 0
            for b in range(BPC):
                tr = tts[b]
                if b + 1 < BPC:
                    xts.append(
                        xpool.tile([CIN, HP, 2, W], FP16, tag="x", name=f"x{b+1}")
                    )
                    tts.append(
                        tpool.tile([CIN, 4, NT, W], FP16, tag="t", name=f"t{b+1}")
                    )
                for tb0, T in BLOCKS:
                    for ct in range(2):
                        co0 = ct * 128
                        # Final block-ct: evacuate m2 alone right after its
                        # group and read m3 straight from PSUM, so the
                        # post-last-matmul chain is one DVE op (short tail).
                        last = b == BPC - 1 and tb0 == BLOCKS[-1][0] and ct == 1
                        msA = psum.tile([128, 2, 4, 128], F32, tag="psA", name="msA", bufs=2)
                        msB = psum.tile([128, 2, 4, 128], F32, tag="psB", name="msB", bufs=2)
                        cas = []
                        for i in range(4):
                            mtile = msA if i < 2 else msB
                            for kw in range(KW):
                                nc.tensor.matmul(
                                    mtile[:, i % 2, 0:T, 0:110],
                                    wr[:, kw, i, co0 : co0 + 128],
                                    tr[:, i, tb0 : tb0 + T, kw : kw + OW],
                                    start=(kw == 0),
                                    stop=(kw == KW - 1),
                                )
                            if i % 2 == 1 and not (last and i == 3):
                                ci = cpool.tile(
                                    [128, 2, TB, OW], FP16, tag=f"ca{i//2}",
                                    name=f"ca{i//2}",
                                )
                                nc.scalar.copy(
                                    ci[:, :, 0:T, :], mtile[:, :, 0:T, 0:110]
                                )
                                cas.append(ci)
                            elif last and i == 2:
                                c2s = cpool.tile(
                                    [128, TB, OW], FP16, tag="c2s", name="c2s"
                                )
                                nc.scalar.copy(
                                    c2s[:, 0:T, :], msB[:, 0, 0:T, 0:110]
                                )
                        c0 = cas[0][:, 0, 0:T, :]
                        c1 = cas[0][:, 1, 0:T, :]
                        if last:
                            c2 = c2s[:, 0:T, :]
                            c3 = msB[:, 1, 0:T, 0:110]
                        else:
                            c2 = cas[1][:, 0, 0:T, :]
                            c3 = cas[1][:, 1, 0:T, :]
                        ts_ = cpool.tile([128, TB, OW], FP16, tag="ts", name="ts_")
                        us_ = cpool.tile([128, TB, OW], FP16, tag="us", name="us_")
                        nc.vector.tensor_tensor(ts_[:, 0:T, :], c1, c2, OP.add)
                        nc.vector.tensor_tensor(us_[:, 0:T, :], c1, c2, OP.subtract)
                        ot = opool.tile([128, TB, 2, OW], FP16, tag="ot", name="ot")
                        cpe = nc.vector.tensor_tensor(
                            ot[:, 0:T, 0, :], ts_[:, 0:T, :], c0, OP.add)
                        nc.vector.tensor_tensor(
                            ot[:, 0:T, 1, :], us_[:, 0:T, :], c3, OP.subtract)
                        nc.sync.dma_start(
                            o_d[b, co0 : co0 + 128, tb0 : tb0 + T, :, :],
                            ot[:, 0:T, :, :])

                        # paced successor-image x streaming
                        bct = gbct - b * N_BCT
                        if b + 1 < BPC and bct % 3 == 0 and bct // 3 < len(XCH):
                            p0, p1 = XCH[bct // 3]
                            dma = nc.gpsimd.dma_start(
                                xts[b + 1][:, p0:p1, :, :],
                                x_d[b + 1, :, p0:p1, :, :],
                            )
                            _add_dep_helper(
                                dma.ins, cpe.ins, sync=True,
                                reason="pace x prefetch vs compute",
                            )
                        # drain up to 2 eligible transform ops
                        popped = 0
                        while pending and popped < 2 and pending[0][2] <= gbct:
                            tb_, tj, _ = pending.popleft()
                            temit(tb_, tj)
                            popped += 1
                        gbct += 1
            # any leftovers (shouldn't happen)
            while pending:
                tb_, tj, _ = pending.popleft()
                temit(tb_, tj)

    nc.compile()
    return nc


def _get_nc():
    if "nc" not in _CACHE:
        _CACHE["nc"] = _build_nc()
    return _CACHE["nc"]


LAST_RESULT = None

_G = np.array(
    [[1, 0, 0], [0.5, 0.5, 0.5], [0.5, -0.5, 0.5], [0, 0, 1]], np.float32
)


def kernel(x, weight, trace=False):
    global LAST_RESULT
    x16 = np.asarray(x, dtype=np.float32).astype(np.float16)
    x16 = x16.reshape(B, CIN, HP, 2, W)
    w32 = np.asarray(weight, dtype=np.float32)
    # Gw[cin, kw, i, cout] = sum_kh G[i, kh] * w[cout, cin, kh, kw]
    gw = np.einsum("ik,ockw->cwio", _G, w32).astype(np.float16)
    gw = np.ascontiguousarray(gw)

    nc = _get_nc()
    in_maps = [
        {"x": x16[i * BPC : (i + 1) * BPC], "w": gw} for i in range(NCORES)
    ]
    res = run_bass_kernel_spmd(
        nc, in_maps, core_ids=list(range(NCORES)), trace=trace
    )
    LAST_RESULT = res
    out = np.concatenate(
        [r["o"].reshape(BPC, COUT, OH, OW) for r in res.results], axis=0
    )
    return out.astype(np.float32)


# revision 8
# speedup vs baseline: 1.0572x; 1.0572x over previous
"""Conv2d 3x3 VALID stride-1 for Trainium2 (Bass/Tile), 8-core SPMD.
1-D Winograd F(2,3) along H: 12 matmuls per 8 output rows instead of 18.

x: [32, 128, 112, 112] f32, weight: [256, 128, 3, 3] f32
out: [32, 256, 110, 110] f32

Per output-row-pair t (55 pairs/image), with d = x rows [2t..2t+3]:
  t0 = d0-d2, t1 = d1+d2, t2 = d2-d1, t3 = d1-d3          (DVE, fp16, 2x)
  m_i = sum_kw Gw[:,kw,i,:] @ t_i[:, t, kw:kw+110]         (PE, 3 taps into PSUM bank i)
  o[2t]   = m0+m1+m2       o[2t+1] = m1-m2-m3              (DVE, all fp16)
m0..m3 live in two 2-bank PSUM tiles; ScalarE evacuates each pair with
one strided copy to fp16 SBUF as soon as its second bank stops, so the
PE never waits on PSUM recycling and the DVE never reads PSUM. Weight
transform Gw = G @ w over kh is precomputed on host; the fp16 output is
upcast to f32 on host. Data-parallel over batch: 4 img/core. H rows are
viewed as [56, 2] (pair, parity) so the stride-2 row reads of the
transform are plain slices.

Measured on 8xNC-v3 (axon): ~266 us NEFF exec on a cool chip (~320 us
when the chip is P0 power-throttled to 5/6 clock); matmul stream runs at
~185.5 ns per 440-free-dim fp16 matmul (~99% of the 183.3 ns roofline).
rel err (Frobenius) 6.1e-4 vs the fp32 jax reference.
"""

from collections import deque

import numpy as np

import concourse.mybir as mybir
import concourse.tile as tile
from concourse import bacc
from concourse.bass_utils import run_bass_kernel_spmd

B, CIN, H, W = 32, 128, 112, 112
COUT, KH, KW = 256, 3, 3
OH, OW = H - KH + 1, W - KW + 1  # 110, 110
NCORES = 8
BPC = B // NCORES  # images per core
HP = H // 2  # 56 row-pairs of input

NT = OH // 2            # 55 output row-pairs per image
TB = 4                  # row-pairs per block (PSUM bank: 4*110=440 <= 512)
BLOCKS = [(i * TB, TB) for i in range(NT // TB)] + [(NT - NT % TB, NT % TB)]
# -> 13 blocks of 4 + 1 block of 3
N_BCT = len(BLOCKS) * 2  # block-cts per image

# input-transform chunking: 8 chunks; a small first chunk (exactly the
# pairs block 0 needs) shortens the startup critical path. Chunk c covers
# output pairs [TCH[c]) and needs input row-pairs up to TCH[c][1]+1.
TCH = [(0, 4)] + [(8 * c - 4, min(8 * c + 4, NT)) for c in range(1, 8)]
# disjoint x DMA chunks (input row-pairs); chunk c covers what transform
# chunk c needs beyond chunk c-1
XCH = [(0, 5)] + [(8 * c - 3, min(8 * c + 5, HP)) for c in range(1, 8)]

# transform op i -> (pair_shift_a, parity_a, pair_shift_b, parity_b, op):
#   t_i[pair t] = x[2t + a] op x[2t + b],  row 2t+d -> (pair t + d//2, d%2)
_TOPS = [
    (0, 0, 1, 0, "subtract"),  # t0 = d0 - d2
    (0, 1, 1, 0, "add"),       # t1 = d1 + d2
    (1, 0, 0, 1, "subtract"),  # t2 = d2 - d1
    (0, 1, 1, 1, "subtract"),  # t3 = d1 - d3
]

F32 = mybir.dt.float32
FP16 = mybir.dt.float16

_CACHE = {}


def _build_nc():
    nc = bacc.Bacc("TRN2", target_bir_lowering=False, debug=False)
    OP = mybir.AluOpType

    x_d = nc.dram_tensor("x", [BPC, CIN, HP, 2, W], FP16, kind="ExternalInput")
    w_d = nc.dram_tensor("w", [CIN, KW, 4, COUT], FP16, kind="ExternalInput")
    # [.., 55, 2, 110]: output row 2t+p lives at [t, p] -> even/odd stores
    # are plain slices
    o_d = nc.dram_tensor("o", [BPC, COUT, NT, 2, OW], FP16, kind="ExternalOutput")

    from concourse.bass import _add_dep_helper

    with tile.TileContext(nc) as tc:
        with (
            tc.tile_pool(name="wpool", bufs=1) as wpool,
            tc.tile_pool(name="xpool", bufs=2) as xpool,
            tc.tile_pool(name="tpool", bufs=2) as tpool,
            tc.tile_pool(name="cpool", bufs=2) as cpool,
            tc.tile_pool(name="opool", bufs=8) as opool,
            tc.tile_pool(name="psum", bufs=8, space="PSUM") as psum,
        ):
            # PE pre-warm on garbage so the HAM clock gate is at 2.4 GHz
            # by the time real matmuls start.
            wscr = wpool.tile([128, 128], FP16, name="warm_w")
            xscr = wpool.tile([128, 4, 110], FP16, name="warm_x")
            nc.vector.memset(wscr[:], 0)
            nc.vector.memset(xscr[:], 0)
            ps_warm = psum.tile([128, 2, 4, 128], F32, name="warm_psum", tag="psA", bufs=2)
            for _ in range(12):
                nc.tensor.matmul(
                    ps_warm[:, 0, 0:4, 0:110], wscr[:], xscr[:],
                    start=True, stop=True, skip_group_check=True,
                )

            wr = wpool.tile([CIN, KW, 4, COUT], FP16)
            xts = [xpool.tile([CIN, HP, 2, W], FP16, tag="x", name="x0")]
            tts = [tpool.tile([CIN, 4, NT, W], FP16, tag="t", name="t0")]
            # x chunk 0 first: it gates the first transform ops + matmuls
            p0, p1 = XCH[0]
            nc.gpsimd.dma_start(xts[0][:, p0:p1, :, :], x_d[0, :, p0:p1, :, :])
            nc.gpsimd.dma_start(wr[:, :, :, 0:128], w_d[:, :, :, 0:128])
            nc.gpsimd.dma_start(wr[:, :, :, 128:256], w_d[:, :, :, 128:256])
            for p0, p1 in XCH[1:]:
                nc.gpsimd.dma_start(
                    xts[0][:, p0:p1, :, :], x_d[0, :, p0:p1, :, :]
                )

            def temit(b, j):
                """Input-transform op j (chunk j//4, i=j%4) for image b."""
                c, i = divmod(j, 4)
                t0, t1 = TCH[c]
                xr, tr = xts[b], tts[b]
                sa, pa, sb, pb, opname = _TOPS[i]
                nc.vector.tensor_tensor(
                    tr[:, i, t0:t1, :],
                    xr[:, t0 + sa : t1 + sa, pa, :],
                    xr[:, t0 + sb : t1 + sb, pb, :],
                    getattr(OP, opname),
                )

            # pending transform ops: (image, op j, min global bct to issue)
            NOPS = 4 * len(TCH)
            pending = deque()
            for j in range(12):
                temit(0, j)  # front-loaded; blocks 0..4 covered
            for j in range(12, NOPS):
                pending.append((0, j, j - 12))
            for b in range(1, BPC):
                for j in range(NOPS):
                    # x chunk c=j//4 of image b is released at global bct
                    # (b-1)*N_BCT + max(0, 3c-3); leave >=7 bcts for the DMA
                    # to land (the in-order DVE stalls the whole pipeline if
                    # a transform op heads its queue before its x rows are
                    # in SBUF). Also spread ops evenly (~1.15/bct): two
                    # transform ops in one bct puts the DVE over the PE
                    # period and the lag chains into PE stalls via the
                    # c-tile WAR.
                    el = max(3 * (j // 4) + 4, 2 + (25 * j) // 31)
                    pending.append((b, j, (b - 1) * N_BCT + el))

            gbct = 0
            for b in range(BPC):
                tr = tts[b]
                if b + 1 < BPC:
                    xts.append(
                        xpool.tile([CIN, HP, 2, W], FP16, tag="x", name=f"x{b+1}")
                    )
                    tts.append(
                        tpool.tile([CIN, 4, NT, W], FP16, tag="t", name=f"t{b+1}")
                    )
                for tb0, T in BLOCKS:
                    for ct in range(2):
                        co0 = ct * 128
                        # Final block-ct: evacuate m2 alone right after its
                        # group and read m3 straight from PSUM, so the
                        # post-last-matmul chain is one DVE op (short tail).
                        last = b == BPC - 1 and tb0 == BLOCKS[-1][0] and ct == 1
                        msA = psum.tile([128, 2, 4, 128], F32, tag="psA", name="msA", bufs=2)
                        msB = psum.tile([128, 2, 4, 128], F32, tag="psB", name="msB", bufs=2)
                        cas = []
                        for i in range(4):
                            mtile = msA if i < 2 else msB
                            for kw in range(KW):
                                nc.tensor.matmul(
                                    mtile[:, i % 2, 0:T, 0:110],
                                    wr[:, kw, i, co0 : co0 + 128],
                                    tr[:, i, tb0 : tb0 + T, kw : kw + OW],
                                    start=(kw == 0),
                                    stop=(kw == KW - 1),
                                )
                            if i % 2 == 1 and not (last and i == 3):
                                ci = cpool.tile(
                                    [128, 2, TB, OW], FP16, tag=f"ca{i//2}",
                                    name=f"ca{i//2}",
                                )
                                nc.scalar.copy(
                                    ci[:, :, 0:T, :], mtile[:, :, 0:T, 0:110]
                                )
                                cas.append(ci)
                            elif last and i == 2:
                                c2s = cpool.tile(
                                    [128, TB, OW], FP16, tag="c2s", name="c2s"
                                )
                                nc.scalar.copy(
                                    c2s[:, 0:T, :], msB[:, 0, 0:T, 0:110]
                                )
                        c0 = cas[0][:, 0, 0:T, :]
                        c1 = cas[0][:, 1, 0:T, :]
                        if last:
                            c2 = c2s[:, 0:T, :]
                            c3 = msB[:, 1, 0:T, 0:110]
                        else:
                            c2 = cas[1][:, 0, 0:T, :]
                            c3 = cas[1][:, 1, 0:T, :]
                        ts_ = cpool.tile([128, TB, OW], FP16, tag="ts", name="ts_")
                        us_ = cpool.tile([128, TB, OW], FP16, tag="us", name="us_")
                        nc.vector.tensor_tensor(ts_[:, 0:T, :], c1, c2, OP.add)
                        nc.vector.tensor_tensor(us_[:, 0:T, :], c1, c2, OP.subtract)
                        ot = opool.tile([128, TB, 2, OW], FP16, tag="ot", name="ot")
                        cpe = nc.vector.tensor_tensor(
                            ot[:, 0:T, 0, :], ts_[:, 0:T, :], c0, OP.add)
                        nc.vector.tensor_tensor(
                            ot[:, 0:T, 1, :], us_[:, 0:T, :], c3, OP.subtract)
                        nc.sync.dma_start(
                            o_d[b, co0 : co0 + 128, tb0 : tb0 + T, :, :],
                            ot[:, 0:T, :, :])

                        # paced successor-image x streaming: chunks 0,1 at
                        # bct 0, then chunk c at bct 3*(c-1)
                        bct = gbct - b * N_BCT
                        if b + 1 < BPC and bct % 3 == 0:
                            chunks = (
                                [0, 1] if bct == 0
                                else [bct // 3 + 1] if bct // 3 + 1 < len(XCH)
                                else []
                            )
                            for c_ in chunks:
                                p0, p1 = XCH[c_]
                                dma = nc.gpsimd.dma_start(
                                    xts[b + 1][:, p0:p1, :, :],
                                    x_d[b + 1, :, p0:p1, :, :],
                                )
                                _add_dep_helper(
                                    dma.ins, cpe.ins, sync=True,
                                    reason="pace x prefetch vs compute",
                                )
                        # drain up to 2 eligible transform ops
                        popped = 0
                        while pending and popped < 2 and pending[0][2] <= gbct:
                            tb_, tj, _ = pending.popleft()
                            temit(tb_, tj)
                            popped += 1
                        gbct += 1
            # any leftovers (shouldn't happen)
            while pending:
                tb_, tj, _ = pending.popleft()
                temit(tb_, tj)

    nc.compile()
    return nc


def _get_nc():
    if "nc" not in _CACHE:
        _CACHE["nc"] = _build_nc()
    return _CACHE["nc"]


LAST_RESULT = None

_G = np.array(
    [[1, 0, 0], [0.5, 0.5, 0.5], [0.5, -0.5, 0.5], [0, 0, 1]], np.float32
)


def kernel(x, weight, trace=False):
    global LAST_RESULT
    x16 = np.asarray(x, dtype=np.float32).astype(np.float16)
    x16 = x16.reshape(B, CIN, HP, 2, W)
    w32 = np.asarray(weight, dtype=np.float32)
    # Gw[cin, kw, i, cout] = sum_kh G[i, kh] * w[cout, cin, kh, kw]
    gw = np.einsum("ik,ockw->cwio", _G, w32).astype(np.float16)
    gw = np.ascontiguousarray(gw)

    nc = _get_nc()
    in_maps = [
        {"x": x16[i * BPC : (i + 1) * BPC], "w": gw} for i in range(NCORES)
    ]
    res = run_bass_kernel_spmd(
        nc, in_maps, core_ids=list(range(NCORES)), trace=trace
    )
    LAST_RESULT = res
    out = np.concatenate(
        [r["o"].reshape(BPC, COUT, OH, OW) for r in res.results], axis=0
    )
    return out.astype(np.float32)
